# revision 6
# baseline (speedup 1.0000x reference)
import os
import sys

sys.path.insert(0, "/opt/trn_rl_repo")
os.environ.setdefault("NEURON_RT_RESET_CORES", "1")

import numpy as np
import ml_dtypes

import concourse.bass as bass
import concourse.bacc as bacc
import concourse.tile as tile
from concourse import mybir

# ---- problem constants (hardcoded; must match reference setup) ----
B, CIN, COUT = 8, 64, 64
E, HEAD, KS = 32, 4, 3
IH = IW = 56
P = IH * IW  # 3136
HP = WP = IH + 2  # padded grid 58x58
PP = HP * WP  # 3364
NCORES = 8
SCALE = float(KS) ** -0.5

F32 = mybir.dt.float32
BF16 = mybir.dt.bfloat16

ROWS_PER_TILE = 2
TPX = ROWS_PER_TILE * WP  # 116 pixels per tile (2 padded rows)
NTILES = IH // ROWS_PER_TILE  # 28

NQK = 1056  # q(512) | k(512) | pe(32) columns per dx
NG = 4096  # G columns per dx: (h, c, d)


def _ap(t, dims):
    """View a pool tile with hand-built free-dim [step, count] pairs."""
    return bass.AP(tensor=t.tensor, offset=t.offset, ap=[list(t.ap[0])] + [list(d) for d in dims])


def _apo(t, n, dims):
    """Like _ap but with an extra element offset."""
    return bass.AP(tensor=t.tensor, offset=t.offset + n, ap=[list(t.ap[0])] + [list(d) for d in dims])


def build_program(n_iters=1):
    nc = bacc.Bacc("TRN2", target_bir_lowering=False)

    x_h = nc.dram_tensor("x", [CIN, P], F32, kind="ExternalInput")
    w_in_t_h = nc.dram_tensor("w_in_t", [CIN, E], F32, kind="ExternalInput")
    wqk_h = nc.dram_tensor("wqk", [96, 3 * NQK], BF16, kind="ExternalInput")
    wg_h = nc.dram_tensor("wg", [96, 3 * NG], BF16, kind="ExternalInput")
    w_out_t_h = nc.dram_tensor("w_out_t", [E, COUT], BF16, kind="ExternalInput")
    ident_h = nc.dram_tensor("ident", [128, 128], F32, kind="ExternalInput")
    out_h = nc.dram_tensor("out", [COUT, P], F32, kind="ExternalOutput")

    from contextlib import ExitStack

    with tile.TileContext(nc) as tc:
        with ExitStack() as ctx:
            stage_pool = ctx.enter_context(tc.tile_pool(name="stage", bufs=1))
            const_pool = ctx.enter_context(tc.tile_pool(name="const", bufs=1))
            persist = ctx.enter_context(tc.tile_pool(name="persist", bufs=1))
            qk_pool = ctx.enter_context(tc.tile_pool(name="qk", bufs=2))
            g_pool = ctx.enter_context(tc.tile_pool(name="gsb", bufs=2))
            u1_pool = ctx.enter_context(tc.tile_pool(name="u1p", bufs=1))
            t2_pool = ctx.enter_context(tc.tile_pool(name="t2p", bufs=2))
            l_pool = ctx.enter_context(tc.tile_pool(name="lp", bufs=2))
            e_pool = ctx.enter_context(tc.tile_pool(name="ep", bufs=2))
            tp_pool = ctx.enter_context(tc.tile_pool(name="tpp", bufs=1))
            scr_pool = ctx.enter_context(tc.tile_pool(name="scr", bufs=1))
            small_pool = ctx.enter_context(tc.tile_pool(name="small", bufs=2))
            ps_qkpe_pool = ctx.enter_context(tc.tile_pool(name="ps_qkpe", bufs=1, space="PSUM"))
            ps_g_pool = ctx.enter_context(tc.tile_pool(name="ps_g", bufs=1, space="PSUM"))
            ps_y_pool = ctx.enter_context(tc.tile_pool(name="ps_y", bufs=1, space="PSUM"))
            ps_o_pool = ctx.enter_context(tc.tile_pool(name="ps_o", bufs=1, space="PSUM"))
            ctx.enter_context(nc.allow_low_precision(reason="bf16 attention pipeline"))
            # ---- inputs: x/w_in_t DMA directly (setup-only consumers);
            # loop-read bf16 weights go via stage + compute copy so loop PE
            # instructions never wait directly on multi-queue DMA sems ----
            x_sb = const_pool.tile([CIN, P], F32, tag="x_c")
            nc.sync.dma_start(out=x_sb, in_=x_h[:, :])
            w_in_t = const_pool.tile([CIN, E], F32, tag="w_in_c")
            nc.sync.dma_start(out=w_in_t, in_=w_in_t_h[:, :])

            def launder_bf16(h, parts, cols):
                dstt = const_pool.tile([parts, cols], BF16, tag=h.name + "_c")
                for j0 in range(0, cols, NG):
                    j1 = min(j0 + NG, cols)
                    stg = stage_pool.tile([128, NG], BF16, tag="stg_b")
                    nc.sync.dma_start(out=stg[:parts, :j1 - j0], in_=h[:, j0:j1])
                    nc.vector.tensor_copy(dstt[:, j0:j1], stg[:parts, :j1 - j0])
                return dstt

            wqk = launder_bf16(wqk_h, 96, 3 * NQK)
            wg = launder_bf16(wg_h, 96, 3 * NG)
            w_out_t = launder_bf16(w_out_t_h, E, COUT)
            ident = const_pool.tile([128, 128], F32, tag="ident_c")
            nc.sync.dma_start(out=ident, in_=ident_h[:, :])

            # ---- xe_sh [96, 3364] bf16: partitions (g, c'), where row
            # g*32+c' holds xe[c'] shifted by (g-1) image rows, zero-padded.
            xe_sh = persist.tile([96, PP], BF16)
            nc.gpsimd.memset(xe_sh, 0.0)
            xe_sh3 = xe_sh.rearrange("p (r w) -> p r w", w=WP)
            for rb in range(7):
                ps_xe = ps_g_pool.tile([E, 448], F32, tag="ps_xe")
                nc.tensor.matmul(
                    ps_xe, w_in_t, x_sb[:, rb * 448:(rb + 1) * 448],
                    start=True, stop=True,
                )
                src = ps_xe.rearrange("p (r w) -> p r w", w=IW)
                for g in range(3):
                    r0 = 8 * rb - g + 2
                    eng = nc.scalar.copy if g == 1 else (
                        lambda out, in_: nc.vector.tensor_copy(out, in_))
                    eng(out=xe_sh3[32 * g:32 * g + 32, r0:r0 + 8, 1:57], in_=src)

            out3 = out_h.rearrange("p (r w) -> p r w", w=IW)

            # ---- main loop over 28 two-row tiles ----
            for _it in range(n_iters):
              for t in range(NTILES):
                f0 = 58 + TPX * t

                # -- PE: q|k|pe conv matmuls (3 dx accumulated in PSUM).
                # Each matmul's output must fit one 2KB PSUM bank (<=512
                # f32 cols); halves land in adjacent banks of one tile so
                # a single copy drains both. --
                ps_qk = ps_qkpe_pool.tile([TPX, 1024], F32, tag="ps_qk")
                ps_pe = ps_qkpe_pool.tile([TPX, 32], F32, tag="ps_pe")
                for dx in range(3):
                    w0 = dx * NQK
                    st, sp = dx == 0, dx == 2
                    lhsT = xe_sh[:, f0 - 1 + dx: f0 - 1 + dx + TPX]
                    nc.tensor.matmul(ps_qk[:, :512], lhsT, wqk[:, w0:w0 + 512], start=st, stop=sp)
                    nc.tensor.matmul(ps_qk[:, 512:], lhsT, wqk[:, w0 + 512:w0 + 1024], start=st, stop=sp)
                    nc.tensor.matmul(ps_pe, lhsT, wqk[:, w0 + 1024:w0 + 1056], start=st, stop=sp)
                qk_sb = qk_pool.tile([TPX, 1024], BF16, tag="qk")
                nc.scalar.copy(out=qk_sb, in_=ps_qk)
                pe_sb = small_pool.tile([TPX, 32], F32, tag="pe")
                nc.scalar.copy(out=pe_sb, in_=ps_pe)

                # -- PE: G conv matmuls (4 heads x 2 bank-halves x 3 dx) --
                g_sb = g_pool.tile([TPX, NG], BF16, tag="g")
                for h in range(HEAD):
                    ps_g = ps_g_pool.tile([TPX, 1024], F32, tag="ps_g")
                    for j in range(2):
                        for dx in range(3):
                            c0 = dx * NG + h * 1024 + j * 512
                            nc.tensor.matmul(
                                ps_g[:, j * 512:(j + 1) * 512],
                                xe_sh[:, f0 - 1 + dx: f0 - 1 + dx + TPX],
                                wg[:, c0: c0 + 512],
                                start=(dx == 0), stop=(dx == 2),
                            )
                    nc.scalar.copy(out=g_sb[:, h * 1024:(h + 1) * 1024], in_=ps_g)

                # -- attention core --
                u1 = u1_pool.tile([TPX, 8192], BF16, tag="u1")
                ll = l_pool.tile([TPX, 4096], BF16, tag="L")
                ee = e_pool.tile([TPX, 4096], BF16, tag="E")
                for hh in range(2):
                    # U1[p,(h2,c,d,k4)] = q * k  (bf16, 2x mode)
                    nc.vector.tensor_mul(
                        _ap(u1, [[4096, 2], [128, 32], [4, 32], [1, 4]]),
                        _apo(qk_sb, 256 * hh, [[128, 2], [4, 32], [0, 32], [1, 4]]),
                        _apo(qk_sb, 512 + 256 * hh, [[128, 2], [0, 32], [4, 32], [1, 4]]),
                    )
                    # pair-sum over k: t2[p,(g,j)] = u1[4g+2j] + u1[4g+2j+1]
                    t2 = t2_pool.tile([TPX, 4096], BF16, tag=f"t2_{hh}")
                    nc.vector.tensor_add(
                        _ap(t2, [[2, 2048], [1, 2]]),
                        _ap(u1, [[4, 2048], [1, 2]]),
                        _apo(u1, 2, [[4, 2048], [1, 2]]),
                    )
                    # final k level on gpsimd: L = t2[2g] + t2[2g+1]
                    nc.gpsimd.tensor_add(
                        _apo(ll, 2048 * hh, [[1, 2048]]),
                        _ap(t2, [[2, 2048]]),
                        _apo(t2, 1, [[2, 2048]]),
                    )
                    # E = exp(scale * L)
                    nc.scalar.activation(
                        out=_apo(ee, 2048 * hh, [[1, 2048]]),
                        in_=_apo(ll, 2048 * hh, [[1, 2048]]),
                        func=mybir.ActivationFunctionType.Exp, scale=SCALE,
                    )

                # -- numerator: T = E * G, S[h,c] = sum_d T (DVE tree) --
                tp = tp_pool.tile([TPX, 4096], BF16, tag="T")
                nc.vector.tensor_mul(tp, ee, g_sb)
                st = scr_pool.tile([TPX, 2048], BF16, tag="st")
                nc.vector.tensor_add(
                    _ap(st, [[16, 128], [1, 16]]),
                    _ap(tp, [[32, 128], [1, 16]]),
                    _apo(tp, 16, [[32, 128], [1, 16]]),
                )
                for w in (8, 4, 2):
                    nc.vector.tensor_add(
                        _ap(st, [[16, 128], [1, w]]),
                        _ap(st, [[16, 128], [1, w]]),
                        _apo(st, w, [[16, 128], [1, w]]),
                    )
                s_sb = small_pool.tile([TPX, 128], F32, tag="S")
                nc.vector.tensor_add(
                    _ap(s_sb, [[1, 128]]),
                    _ap(st, [[16, 128]]),
                    _apo(st, 1, [[16, 128]]),
                )

                # -- denominator: Z[h,c] = sum_d E (gpsimd tree) --
                zt = scr_pool.tile([TPX, 2048], BF16, tag="zt")
                nc.gpsimd.tensor_add(
                    _ap(zt, [[16, 128], [1, 16]]),
                    _ap(ee, [[32, 128], [1, 16]]),
                    _apo(ee, 16, [[32, 128], [1, 16]]),
                )
                for w in (8, 4, 2):
                    nc.gpsimd.tensor_add(
                        _ap(zt, [[16, 128], [1, w]]),
                        _ap(zt, [[16, 128], [1, w]]),
                        _apo(zt, w, [[16, 128], [1, w]]),
                    )
                z_sb = small_pool.tile([TPX, 128], F32, tag="Z")
                nc.gpsimd.tensor_add(
                    _ap(z_sb, [[1, 128]]),
                    _ap(zt, [[16, 128]]),
                    _apo(zt, 1, [[16, 128]]),
                )

                # -- y[c] = sum_h S[h,c]/Z[h,c], + pe --
                r_sb = small_pool.tile([TPX, 128], F32, tag="R")
                nc.vector.reciprocal_approx_fast(out=r_sb[:, :], in_=z_sb[:, :])
                yt = small_pool.tile([TPX, 128], F32, tag="ytt")
                nc.vector.tensor_mul(yt, r_sb, s_sb)
                nc.vector.tensor_add(
                    _ap(yt, [[1, 64]]), _ap(yt, [[1, 64]]), _apo(yt, 64, [[1, 64]]),
                )
                y32 = small_pool.tile([TPX, 32], F32, tag="y32")
                nc.vector.tensor_add(
                    _ap(y32, [[1, 32]]),
                    _ap(yt, [[1, 32]]),
                    _apo(yt, 32, [[1, 32]]),
                )
                yp_sb = small_pool.tile([TPX, 32], F32, tag="yp")
                nc.vector.tensor_add(yp_sb, y32, pe_sb)

                # -- outProj --
                ps_yt = ps_y_pool.tile([E, TPX], F32, tag="ps_yt")
                nc.tensor.transpose(ps_yt, yp_sb, ident[:TPX, :TPX])
                yT = small_pool.tile([E, TPX], BF16, tag="yT")
                nc.scalar.copy(out=yT, in_=ps_yt)
                ps_o = ps_o_pool.tile([COUT, TPX], F32, tag="ps_o")
                nc.tensor.matmul(ps_o, w_out_t, yT, start=True, stop=True)
                o_sb = small_pool.tile([COUT, TPX], F32, tag="o_sb")
                nc.scalar.copy(out=o_sb, in_=ps_o)

                src = o_sb.rearrange("p (r w) -> p r w", w=WP)
                nc.sync.dma_start(
                    out=out3[:, ROWS_PER_TILE * t: ROWS_PER_TILE * (t + 1), :],
                    in_=src[:, :, 1:57],
                )

    if not nc.is_finalized():
        nc.finalize()
    return nc


def _bf16(a):
    return np.asarray(a, np.float32).astype(ml_dtypes.bfloat16)


def _prep_weights(w_in, w_q, w_k, w_v, w_pe, w_p1, w_out):
    w_q = np.asarray(w_q, np.float32)
    w_k = np.asarray(w_k, np.float32)
    w_v = np.asarray(w_v, np.float32)
    w_pe = np.asarray(w_pe, np.float32)
    w_p1 = np.asarray(w_p1, np.float32)

    # wqk[(dy,c'), (dx, [q|k|pe])]
    wqk = np.zeros((3, 3, 32, NQK), np.float32)  # [dy, dx, c', col]
    for h in range(HEAD):
        for k in range(KS):
            for c in range(E):
                oc = c * (HEAD * KS) + h * KS + k
                # q block: col (h, c, k4); contraction row c'=c
                wqk[:, :, c, h * 128 + c * 4 + k] = w_q[oc, 0, :, :]
                # k block: col (h, d, k4); source channel d=c
                wqk[:, :, c, 512 + h * 128 + c * 4 + k] = w_k[oc, 0, :, :]
    for e in range(E):
        wqk[:, :, e, 1024 + e] = w_pe[e, 0, :, :]
    # -> [96=(dy,c'), 3*NQK=(dx, col)]
    wqk = wqk.transpose(0, 2, 1, 3).reshape(96, 3 * NQK)

    # WG[(dy,d'), (dx, (h,c,d))]: W2[c,h,d,dy,dx] = sum_k p1[c,h*3+k]*wv[d*12+h*3+k,0,dy,dx]
    wgm = np.zeros((3, 3, 32, NG), np.float32)  # [dy, dx, d', col]
    for h in range(HEAD):
        for d in range(E):
            vv = w_v[d * (HEAD * KS) + h * KS: d * (HEAD * KS) + h * KS + KS, 0, :, :]
            for c in range(E):
                w2 = np.einsum("k,kyx->yx", w_p1[c, h * KS:(h + 1) * KS], vv)
                wgm[:, :, d, h * 1024 + c * 32 + d] = w2
    wg = wgm.transpose(0, 2, 1, 3).reshape(96, 3 * NG)

    return {
        "w_in_t": np.ascontiguousarray(np.asarray(w_in, np.float32).T),
        "wqk": _bf16(wqk),
        "wg": _bf16(wg),
        "w_out_t": _bf16(np.asarray(w_out, np.float32).T),
        "ident": np.eye(128, dtype=np.float32),
    }


_NC_CACHE = {}


def kernel(x, w_in, w_q, w_k, w_v, w_pe, w_p1, w_out):
    from concourse.bass_utils import run_bass_kernel_spmd

    x = np.asarray(x, np.float32)
    weights = _prep_weights(w_in, w_q, w_k, w_v, w_pe, w_p1, w_out)
    if "nc" not in _NC_CACHE:
        _NC_CACHE["nc"] = build_program()
    nc = _NC_CACHE["nc"]

    in_maps = []
    for i in range(NCORES):
        m = dict(weights)
        m["x"] = np.ascontiguousarray(x[i].reshape(CIN, P))
        in_maps.append(m)

    res = run_bass_kernel_spmd(nc, in_maps, list(range(NCORES)))
    outs = [res.results[i]["out"].reshape(COUT, IH, IW) for i in range(NCORES)]
    return np.stack(outs, axis=0)


if __name__ == "__main__":
    nc = build_program()
    print("program built ok")
